# revision 1
# baseline (speedup 1.0000x reference)
import sys

sys.path.insert(0, "/opt/trn_rl_repo")
import numpy as np
import concourse.bass as bass  # noqa: F401
import concourse.mybir as mybir
import concourse.tile as tile
from concourse import bacc
from concourse.bass_utils import run_bass_kernel_spmd

B, T, C, H, D = 4, 2048, 2048, 16, 128
NCORES = 8
HPC = H // NCORES  # heads per core
F = HPC * D  # 256 per-core features
TOK = B * T  # 8192
CC = C // 128  # 16 contraction chunks
NTB = TOK // 512  # 16 token blocks
TOKPC = TOK // NCORES  # 1024 tokens per core for out-proj

f32 = mybir.dt.float32
f32r = mybir.dt.float32r
SCALE = 1.0 / float(np.sqrt(D))
EXP_BIAS = -8.0  # constant shift inside exp; cancels in normalization

USE_F32R = True
PROJ_PHASE = True
ATTN_PHASE = True


def _mm(ap):
    return ap


def _build_launch_a():
    nc = bacc.Bacc("TRN2", target_bir_lowering=False, debug=False)
    xT = nc.dram_tensor("xT", [C, TOK], f32r, kind="ExternalInput")
    wqkT = nc.dram_tensor("wqkT", [C, 2 * F], f32r, kind="ExternalInput")
    wvT = nc.dram_tensor("wvT", [C, F], f32r, kind="ExternalInput")
    bqk = nc.dram_tensor("bqk", [2 * F, 1], f32, kind="ExternalInput")
    bv = nc.dram_tensor("bv", [F, 1], f32, kind="ExternalInput")
    ones_in = nc.dram_tensor("ones_in", [128, 1], f32r, kind="ExternalInput")
    attvT = nc.dram_tensor("attvT", [F, TOK], f32, kind="ExternalOutput")
    qkT = nc.dram_tensor("qkT", [2 * F, TOK], f32r)  # rows 0:256 qT, 256:512 kT
    vsc = nc.dram_tensor("vsc", [TOK, F], f32r)  # V natural [tok, feat]

    with tile.TileContext(nc) as tc:
        with tc.tile_pool(name="const", bufs=1) as cpool:
            wqk_sb = cpool.tile([128, CC, 2 * F], f32r)
            wv_sb = cpool.tile([128, CC, F], f32r)
            bqk_sb = cpool.tile([128, 4, 1], f32)
            bv_sb = cpool.tile([128, HPC, 1], f32)
            ones_sb = cpool.tile([128, 1], f32r)
            ones1_sb = cpool.tile([1, 128], f32)
            ebias_sb = cpool.tile([128, 1], f32)
            nc.vector.memset(ebias_sb[:], EXP_BIAS)
            nc.sync.dma_start(
                out=wqk_sb[:], in_=wqkT[:].rearrange("(cc p) f -> p cc f", p=128)
            )
            nc.sync.dma_start(
                out=wv_sb[:], in_=wvT[:].rearrange("(cc p) f -> p cc f", p=128)
            )
            nc.sync.dma_start(
                out=bqk_sb[:], in_=bqk[:].rearrange("(fb p) o -> p fb o", p=128)
            )
            nc.sync.dma_start(
                out=bv_sb[:], in_=bv[:].rearrange("(h p) o -> p h o", p=128)
            )
            nc.sync.dma_start(out=ones_sb[:], in_=ones_in[:])
            nc.vector.memset(ones1_sb[:], 1.0)

            # ---- Phase 1: QKV projections ----
            with (
                tc.tile_pool(name="px", bufs=2) as px_pool,
                tc.tile_pool(name="pq", bufs=3) as pq_pool,
                tc.tile_pool(name="psA", bufs=3, space="PSUM") as psA,
                tc.tile_pool(name="psV", bufs=2, space="PSUM") as psV,
            ):
                for tb in range(NTB if PROJ_PHASE else 0):
                    xt = px_pool.tile([128, CC, 512], f32r, tag="xt", name=f"xt{tb}")
                    nc.sync.dma_start(
                        out=xt[:],
                        in_=xT[:, tb * 512 : (tb + 1) * 512].rearrange(
                            "(cc p) t -> p cc t", p=128
                        ),
                    )
                    # Q^T,K^T: [feat_block 128, tok 512], weights stationary
                    for fb in range(4):
                        ps = psA.tile(
                            [128, 512], f32, tag="psA", name=f"psA{tb}_{fb}"
                        )
                        for cc in range(CC):
                            nc.tensor.matmul(
                                ps[:],
                                _mm(wqk_sb[:, cc, fb * 128 : (fb + 1) * 128]),
                                _mm(xt[:, cc]),
                                start=(cc == 0),
                                stop=(cc == CC - 1),
                            )
                        qko = pq_pool.tile([128, 512], f32r, tag="qko", name=f"qko{tb}_{fb}")
                        nc.vector.tensor_scalar_add(qko[:], ps[:], bqk_sb[:, fb])
                        nc.sync.dma_start(
                            out=qkT[fb * 128 : (fb + 1) * 128, tb * 512 : (tb + 1) * 512],
                            in_=qko[:],
                        )
                    # V natural: [tok_sub 128, feat 256], x^T tiles stationary
                    for sub in range(4):
                        psv = psV.tile([128, F], f32, tag="psV", name=f"psV{tb}_{sub}")
                        for cc in range(CC):
                            nc.tensor.matmul(
                                psv[:],
                                _mm(xt[:, cc, sub * 128 : (sub + 1) * 128]),
                                _mm(wv_sb[:, cc]),
                                start=(cc == 0),
                                stop=(cc == CC - 1),
                            )
                        vo = pq_pool.tile([128, F], f32r, tag="vo", name=f"vo{tb}_{sub}")
                        nc.vector.tensor_copy(vo[:], psv[:])
                        r0 = tb * 512 + sub * 128
                        nc.sync.dma_start(out=vsc[r0 : r0 + 128, :], in_=vo[:])

            # ---- Phase 2: attention (S^T layout, per (b, head)) ----
            with (
                tc.tile_pool(name="kv", bufs=2) as kv_pool,
                tc.tile_pool(name="qp", bufs=2) as q_pool,
                tc.tile_pool(name="pt", bufs=2) as pt_pool,
                tc.tile_pool(name="ao", bufs=2) as ao_pool,
                tc.tile_pool(name="ps_st", bufs=3, space="PSUM") as ps_st_pool,
                tc.tile_pool(name="ps_av", bufs=2, space="PSUM") as ps_av_pool,
                tc.tile_pool(name="ps_sum", bufs=2, space="PSUM") as ps_sum_pool,
                tc.tile_pool(name="ps_bc", bufs=1, space="PSUM") as ps_bc_pool,
            ):
                for b in range(B if ATTN_PHASE else 0):
                    for h in range(HPC):
                        kT_sb = kv_pool.tile([128, T], f32r, tag="kT", name=f"kT{b}_{h}")
                        nc.sync.dma_start(
                            out=kT_sb[:],
                            in_=qkT[F + h * 128 : F + (h + 1) * 128, b * T : (b + 1) * T],
                        )
                        v_sb = kv_pool.tile(
                            [128, T // 128, 128], f32r, tag="v", name=f"v{b}_{h}"
                        )
                        nc.sync.dma_start(
                            out=v_sb[:],
                            in_=vsc[b * T : (b + 1) * T, h * 128 : (h + 1) * 128].rearrange(
                                "(kb p) d -> p kb d", p=128
                            ),
                        )
                        for qb in range(4):  # 512-wide query blocks
                            qT_sb = q_pool.tile(
                                [128, 512], f32r, tag="qT", name=f"qT{b}_{h}_{qb}"
                            )
                            q0 = b * T + qb * 512
                            nc.sync.dma_start(
                                out=qT_sb[:],
                                in_=qkT[h * 128 : (h + 1) * 128, q0 : q0 + 512],
                            )
                            nkb = (qb + 1) * 4
                            pts = []
                            for kb in range(nkb):
                                ps_st = ps_st_pool.tile(
                                    [128, 512], f32, tag="st", name=f"st{b}_{h}_{qb}_{kb}"
                                )
                                nc.tensor.matmul(
                                    ps_st[:],
                                    _mm(kT_sb[:, kb * 128 : (kb + 1) * 128]),
                                    _mm(qT_sb[:]),
                                    start=True,
                                    stop=True,
                                )
                                pt = pt_pool.tile(
                                    [128, 512], f32r, tag=f"pt{kb}", name=f"pt{b}_{h}_{qb}_{kb}"
                                )
                                nc.scalar.activation(
                                    pt[:],
                                    ps_st[:],
                                    mybir.ActivationFunctionType.Exp,
                                    bias=ebias_sb[:],
                                    scale=SCALE,
                                )
                                if kb * 128 >= qb * 512:
                                    # diagonal block: zero where k_global > q_global
                                    # keep when (qb*512 + q) - (kb*128 + k) >= 0
                                    nc.gpsimd.affine_select(
                                        out=pt[:],
                                        in_=pt[:],
                                        compare_op=mybir.AluOpType.is_ge,
                                        fill=0.0,
                                        base=qb * 512 - kb * 128,
                                        pattern=[[1, 512]],
                                        channel_multiplier=-1,
                                    )
                                pts.append(pt)
                            ps_av = ps_av_pool.tile(
                                [128, 512], f32, tag="av", name=f"av{b}_{h}_{qb}"
                            )
                            ps_sum = ps_sum_pool.tile(
                                [1, 512], f32, tag="sum", name=f"sum{b}_{h}_{qb}"
                            )
                            for kb in range(nkb):
                                nc.tensor.matmul(
                                    ps_av[:],
                                    _mm(v_sb[:, kb]),
                                    _mm(pts[kb][:]),
                                    start=(kb == 0),
                                    stop=(kb == nkb - 1),
                                )
                                nc.tensor.matmul(
                                    ps_sum[:],
                                    _mm(ones_sb[:]),
                                    _mm(pts[kb][:]),
                                    start=(kb == 0),
                                    stop=(kb == nkb - 1),
                                )
                            recip = ao_pool.tile(
                                [1, 512], f32, tag="recip", name=f"rc{b}_{h}_{qb}"
                            )
                            nc.vector.reciprocal(recip[:], ps_sum[:])
                            ps_bc = ps_bc_pool.tile(
                                [128, 512], f32, tag="bc", name=f"bc{b}_{h}_{qb}"
                            )
                            nc.tensor.matmul(
                                ps_bc[:], ones1_sb[:], recip[:], start=True, stop=True
                            )
                            bc_sb = ao_pool.tile(
                                [128, 512], f32, tag="bc_sb", name=f"bcs{b}_{h}_{qb}"
                            )
                            nc.vector.tensor_copy(bc_sb[:], ps_bc[:])
                            out_sb = ao_pool.tile(
                                [128, 512], f32, tag="out_sb", name=f"os{b}_{h}_{qb}"
                            )
                            nc.vector.tensor_mul(out_sb[:], ps_av[:], bc_sb[:])
                            nc.vector.tensor_scalar_add(out_sb[:], out_sb[:], bv_sb[:, h])
                            nc.sync.dma_start(
                                out=attvT[h * 128 : (h + 1) * 128, q0 : q0 + 512],
                                in_=out_sb[:],
                            )
    nc.compile()
    return nc


def _build_launch_b():
    nc = bacc.Bacc("TRN2", target_bir_lowering=False, debug=False)
    avT = nc.dram_tensor("avT", [C, TOKPC], f32r, kind="ExternalInput")
    woT = nc.dram_tensor("woT", [C, C], f32r, kind="ExternalInput")
    bo2 = nc.dram_tensor("bo2", [1, C], f32, kind="ExternalInput")
    outp = nc.dram_tensor("outp", [TOKPC, C], f32, kind="ExternalOutput")

    NB = C // 512  # 4 output-column blocks
    NTB2 = TOKPC // 128  # 8 token blocks

    with tile.TileContext(nc) as tc:
        with tc.tile_pool(name="const", bufs=1) as cpool:
            av_sb = cpool.tile([128, CC, TOKPC], f32r)  # 64 KB/part, resident
            ones1_sb = cpool.tile([1, 128], f32)
            bo_sb = cpool.tile([1, C], f32)
            bias_sb = cpool.tile([128, NB, 512], f32)
            nc.sync.dma_start(
                out=av_sb[:], in_=avT[:].rearrange("(cc p) t -> p cc t", p=128)
            )
            nc.sync.dma_start(out=bo_sb[:], in_=bo2[:])
            nc.vector.memset(ones1_sb[:], 1.0)
            with (
                tc.tile_pool(name="w", bufs=2) as w_pool,
                tc.tile_pool(name="o", bufs=3) as o_pool,
                tc.tile_pool(name="ps", bufs=4, space="PSUM") as ps_pool,
                tc.tile_pool(name="psb", bufs=2, space="PSUM") as psb_pool,
            ):
                for nb in range(NB):
                    ps_b = psb_pool.tile([128, 512], f32, tag="psb", name=f"psb{nb}")
                    nc.tensor.matmul(
                        ps_b[:],
                        ones1_sb[:],
                        bo_sb[:, nb * 512 : (nb + 1) * 512],
                        start=True,
                        stop=True,
                    )
                    nc.vector.tensor_copy(bias_sb[:, nb], ps_b[:])
                for nb in range(NB):
                    wt = w_pool.tile([128, CC, 512], f32r, tag="wt", name=f"wt{nb}")
                    nc.sync.dma_start(
                        out=wt[:],
                        in_=woT[:, nb * 512 : (nb + 1) * 512].rearrange(
                            "(cc p) n -> p cc n", p=128
                        ),
                    )
                    for tb in range(NTB2):
                        ps = ps_pool.tile([128, 512], f32, tag="ps", name=f"ps{nb}_{tb}")
                        for cc in range(CC):
                            nc.tensor.matmul(
                                ps[:],
                                _mm(av_sb[:, cc, tb * 128 : (tb + 1) * 128]),
                                _mm(wt[:, cc]),
                                start=(cc == 0),
                                stop=(cc == CC - 1),
                            )
                        ot = o_pool.tile([128, 512], f32, tag="ot", name=f"ot{nb}_{tb}")
                        nc.vector.tensor_add(ot[:], ps[:], bias_sb[:, nb])
                        nc.sync.dma_start(
                            out=outp[tb * 128 : (tb + 1) * 128, nb * 512 : (nb + 1) * 512],
                            in_=ot[:],
                        )
    nc.compile()
    return nc


_NC_CACHE = {}


def _get_ncs():
    if "a" not in _NC_CACHE:
        _NC_CACHE["a"] = _build_launch_a()
        _NC_CACHE["b"] = _build_launch_b()
    return _NC_CACHE["a"], _NC_CACHE["b"]


def kernel(x, wq, bq, wk, bk, wv, bv, wo, bo):
    x = np.asarray(x, dtype=np.float32)
    nca, ncb = _get_ncs()
    core_ids = list(range(NCORES))

    xT = np.ascontiguousarray(x.reshape(TOK, C).T)
    in_maps_a = []
    for c in range(NCORES):
        hs = slice(c * F, (c + 1) * F)
        wqkT_c = np.ascontiguousarray(
            np.concatenate([np.asarray(wq)[hs], np.asarray(wk)[hs]], axis=0).T
        )
        wvT_c = np.ascontiguousarray(np.asarray(wv)[hs].T)
        bqk_c = np.ascontiguousarray(
            np.concatenate([np.asarray(bq)[hs], np.asarray(bk)[hs]])[:, None]
        )
        bv_c = np.ascontiguousarray(np.asarray(bv)[hs][:, None])
        in_maps_a.append(
            {"xT": xT, "wqkT": wqkT_c, "wvT": wvT_c, "bqk": bqk_c, "bv": bv_c,
             "ones_in": np.ones((128, 1), dtype=np.float32)}
        )
    res_a = run_bass_kernel_spmd(nca, in_maps_a, core_ids)
    attv_full = np.concatenate(
        [res_a.results[c]["attvT"] for c in range(NCORES)], axis=0
    )  # [C, TOK], rows head-major

    woT_np = np.ascontiguousarray(np.asarray(wo).T)
    bo2_np = np.ascontiguousarray(np.asarray(bo)[None, :])
    in_maps_b = []
    for c in range(NCORES):
        av_c = np.ascontiguousarray(attv_full[:, c * TOKPC : (c + 1) * TOKPC])
        in_maps_b.append({"avT": av_c, "woT": woT_np, "bo2": bo2_np})
    res_b = run_bass_kernel_spmd(ncb, in_maps_b, core_ids)
    out = np.concatenate([res_b.results[c]["outp"] for c in range(NCORES)], axis=0)
    return out.reshape(B, T, C)



# revision 2
# speedup vs baseline: 1.0270x; 1.0270x over previous
import sys

sys.path.insert(0, "/opt/trn_rl_repo")
import numpy as np
import ml_dtypes
import concourse.bass as bass  # noqa: F401
import concourse.mybir as mybir
import concourse.tile as tile
from concourse import bacc
from concourse.bass_utils import run_bass_kernel_spmd

B, T, C, H, D = 4, 2048, 2048, 16, 128
NCORES = 8
HPC = H // NCORES  # 2 heads per core
F = HPC * D  # 256 per-core head features
TOK = B * T  # 8192
CC = C // 128  # 16 contraction chunks for the projections
TBPB = T // 512  # 4 token blocks (512-wide) per batch
KBPB = T // 128  # 16 k blocks (128-wide) per batch
TPC = T // NCORES  # 256 tokens of each batch owned per core after RS

f32 = mybir.dt.float32
bf16 = mybir.dt.bfloat16
SCALE = 1.0 / float(np.sqrt(D))

bfnp = ml_dtypes.bfloat16


def _build(do_attn=True, do_outproj=True, do_rs=True):
    nc = bacc.Bacc(
        "TRN2", target_bir_lowering=False, debug=False, num_devices=NCORES
    )
    xT = nc.dram_tensor("xT", [C, TOK], bf16, kind="ExternalInput")
    wqkT = nc.dram_tensor("wqkT", [C, 2 * F], bf16, kind="ExternalInput")
    wvT = nc.dram_tensor("wvT", [C, F], bf16, kind="ExternalInput")
    wosT = nc.dram_tensor("wosT", [F, C], bf16, kind="ExternalInput")
    bqk = nc.dram_tensor("bqk", [2 * F, 1], f32, kind="ExternalInput")
    outp = nc.dram_tensor("outp", [B, TPC, C], bf16, kind="ExternalOutput")

    with tile.TileContext(nc) as tc:
        with (
            tc.tile_pool(name="const", bufs=1) as cpool,
            tc.tile_pool(name="dram", bufs=2, space="DRAM") as dpool,
            tc.tile_pool(name="dramo", bufs=3, space="DRAM") as dopool,
        ):
            wqk_sb = cpool.tile([128, CC, 2 * F], bf16)
            wv_sb = cpool.tile([128, CC, F], bf16)
            wos_sb = cpool.tile([128, HPC, C], bf16)
            bqk_sb = cpool.tile([128, 4, 1], f32)
            ones_sb = cpool.tile([128, 1], bf16)
            ones1_sb = cpool.tile([1, 128], bf16)
            nc.vector.memset(ones_sb[:], 1.0)
            nc.vector.memset(ones1_sb[:], 1.0)
            nc.sync.dma_start(
                out=wqk_sb[:], in_=wqkT[:].rearrange("(cc p) f -> p cc f", p=128)
            )
            nc.sync.dma_start(
                out=bqk_sb[:], in_=bqk[:].rearrange("(fb p) o -> p fb o", p=128)
            )
            nc.gpsimd.dma_start(
                out=wv_sb[:], in_=wvT[:].rearrange("(cc p) f -> p cc f", p=128)
            )
            nc.gpsimd.dma_start(
                out=wos_sb[:], in_=wosT[:].rearrange("(h p) j -> p h j", p=128)
            )

            with (
                tc.tile_pool(name="px", bufs=3) as px_pool,
                tc.tile_pool(name="qkv", bufs=2) as qkv_pool,
                tc.tile_pool(name="attv", bufs=1) as attv_pool,
                tc.tile_pool(name="pt", bufs=2) as pt_pool,
                tc.tile_pool(name="misc", bufs=3) as misc_pool,
                tc.tile_pool(name="oo", bufs=3) as oo_pool,
                tc.tile_pool(name="psA", bufs=2, space="PSUM") as psA,
                tc.tile_pool(name="psV", bufs=1, space="PSUM") as psV,
                tc.tile_pool(name="ps_st", bufs=2, space="PSUM") as ps_st_pool,
                tc.tile_pool(name="ps_av", bufs=2, space="PSUM") as ps_av_pool,
                tc.tile_pool(name="ps_sum", bufs=1, space="PSUM") as ps_sum_pool,
            ):
                # 4 causal-diagonal masks: delta = kb*128 - qb*512 in {0,128,256,384};
                # mask[kp, qf] = 1 where qf >= kp + delta else 0.
                masks = cpool.tile([128, 4, 512], bf16)
                nc.vector.memset(masks[:], 1.0)
                for mi in range(4):
                    nc.gpsimd.affine_select(
                        out=masks[:, mi],
                        in_=masks[:, mi],
                        compare_op=mybir.AluOpType.is_ge,
                        fill=0.0,
                        base=-mi * 128,
                        pattern=[[1, 512]],
                        channel_multiplier=-1,
                    )
                def load_xt(b, t4, engine):
                    tb = b * TBPB + t4
                    xt = px_pool.tile([128, CC, 512], bf16, tag="xt", name=f"xt{tb}")
                    engine.dma_start(
                        out=xt[:],
                        in_=xT[:, tb * 512 : (tb + 1) * 512].rearrange(
                            "(cc p) t -> p cc t", p=128
                        ),
                    )
                    return xt

                xt_pref = [load_xt(0, t4, nc.sync) for t4 in range(TBPB)]

                for b in range(B):
                    qT_sb = qkv_pool.tile([128, HPC, T], bf16, tag="qT", name=f"qT{b}")
                    kT_sb = qkv_pool.tile([128, HPC, T], bf16, tag="kT", name=f"kT{b}")
                    v_sb = qkv_pool.tile([128, KBPB, F], bf16, tag="v", name=f"v{b}")
                    attv_sb = attv_pool.tile(
                        [128, HPC, T], bf16, tag="attv", name=f"attv{b}"
                    )
                    def proj_block(t4):
                        tb = b * TBPB + t4
                        xt = xt_pref[t4]
                        for fb in range(4):
                            ps = psA.tile([128, 512], f32, tag="psA", name=f"psA{tb}_{fb}")
                            for cc in range(CC):
                                nc.tensor.matmul(
                                    ps[:],
                                    wqk_sb[:, cc, fb * 128 : (fb + 1) * 128],
                                    xt[:, cc],
                                    start=(cc == 0),
                                    stop=(cc == CC - 1),
                                )
                            dst = qT_sb if fb < 2 else kT_sb
                            h = fb % 2
                            nc.vector.tensor_scalar_add(
                                dst[:, h, t4 * 512 : (t4 + 1) * 512],
                                ps[:],
                                bqk_sb[:, fb],
                            )
                            # V group interleaved after each QK group: the QK
                            # matmuls hide the psV-copy latency (bufs=1)
                            sub = fb
                            psv = psV.tile([128, F], f32, tag="psV", name=f"psV{tb}_{sub}")
                            for cc in range(CC):
                                nc.tensor.matmul(
                                    psv[:],
                                    xt[:, cc, sub * 128 : (sub + 1) * 128],
                                    wv_sb[:, cc],
                                    start=(cc == 0),
                                    stop=(cc == CC - 1),
                                )
                            nc.vector.tensor_copy(v_sb[:, t4 * 4 + sub], psv[:])

                    def issue_scores(h, qb):
                        nkb = (qb + 1) * 4
                        q0 = qb * 512
                        pts = []
                        for kb in range(nkb):
                            ps_st = ps_st_pool.tile(
                                [128, 512], f32, tag="st", name=f"st{b}_{h}_{qb}_{kb}"
                            )
                            nc.tensor.matmul(
                                ps_st[:],
                                kT_sb[:, h, kb * 128 : (kb + 1) * 128],
                                qT_sb[:, h, q0 : q0 + 512],
                                start=True,
                                stop=True,
                            )
                            pt = pt_pool.tile(
                                [128, 512], bf16, tag=f"pt{kb}",
                                name=f"pt{b}_{h}_{qb}_{kb}",
                            )
                            nc.scalar.activation(
                                pt[:],
                                ps_st[:],
                                mybir.ActivationFunctionType.Exp,
                                scale=SCALE,
                            )
                            if kb * 128 >= q0:
                                # diagonal: zero where k_global > q_global
                                mi = (kb * 128 - q0) // 128
                                nc.vector.tensor_mul(
                                    pt[:], pt[:], masks[:, mi]
                                )
                            pts.append(pt)
                        return (h, qb, pts)

                    def issue_av(pend):
                        h, qb, pts = pend
                        nkb = len(pts)
                        q0 = qb * 512
                        ps_av = ps_av_pool.tile(
                            [128, 512], f32, tag="av", name=f"av{b}_{h}_{qb}"
                        )
                        ps_sum = ps_sum_pool.tile(
                            [1, 512], f32, tag="sum", name=f"sum{b}_{h}_{qb}"
                        )
                        for kb in range(nkb):
                            nc.tensor.matmul(
                                ps_av[:],
                                v_sb[:, kb, h * 128 : (h + 1) * 128],
                                pts[kb][:],
                                start=(kb == 0),
                                stop=(kb == nkb - 1),
                                skip_group_check=True,
                            )
                            nc.tensor.matmul(
                                ps_sum[:],
                                ones_sb[:],
                                pts[kb][:],
                                start=(kb == 0),
                                stop=(kb == nkb - 1),
                                skip_group_check=True,
                            )
                        recip = misc_pool.tile(
                            [1, 512], bf16, tag="recip", name=f"rc{b}_{h}_{qb}"
                        )
                        with nc.allow_low_precision(
                            reason="bf16 softmax denominators are ample"
                        ):
                            nc.vector.reciprocal(recip[:], ps_sum[:])
                        bc_sb = misc_pool.tile(
                            [128, 512], bf16, tag="bc", name=f"bcs{b}_{h}_{qb}"
                        )
                        nc.gpsimd.partition_broadcast(bc_sb[:], recip[:])
                        nc.vector.tensor_mul(
                            attv_sb[:, h, q0 : q0 + 512], ps_av[:], bc_sb[:]
                        )

                    for t4 in range(TBPB):
                        proj_block(t4)
                    # prefetch next batch's x tiles while attention runs
                    if b + 1 < B:
                        xt_pref = [
                            load_xt(b + 1, t, nc.gpsimd) for t in range(TBPB)
                        ]
                    pend = None
                    for h in range(HPC if do_attn else 0):
                        for qb in range(TBPB):
                            nxt = issue_scores(h, qb)
                            if pend is not None:
                                issue_av(pend)
                            pend = nxt
                    if pend is not None:
                        issue_av(pend)
                    if not do_outproj:
                        continue

                    # ---- partial out-projection for batch b ----
                    # PSUM rotates over psA(2)+ps_av(1) banks; epilogue copies
                    # alternate DVE/ACT so neither engine paces the PE.
                    partial = dpool.tile([T, C], bf16, tag="part", name=f"part{b}")
                    for ts_ in range(KBPB):
                        ot = oo_pool.tile(
                            [128, C], bf16, tag="ot", name=f"ot{b}_{ts_}"
                        )
                        for jb in range(4):
                            i = ts_ * 4 + jb
                            # skip the av pool early on: its banks are still
                            # cycling through the tail of the attention phase
                            use_av = i >= 6 and i % 3 == 2
                            pool = ps_av_pool if use_av else psA
                            tag = "av" if use_av else "psA"
                            ps = pool.tile(
                                [128, 512], f32, tag=tag, name=f"po{b}_{ts_}_{jb}"
                            )
                            for h in range(HPC):
                                nc.tensor.matmul(
                                    ps[:],
                                    attv_sb[:, h, ts_ * 128 : (ts_ + 1) * 128],
                                    wos_sb[:, h, jb * 512 : (jb + 1) * 512],
                                    start=(h == 0),
                                    stop=(h == HPC - 1),
                                )
                            dst = ot[:, jb * 512 : (jb + 1) * 512]
                            if i % 2 == 0:
                                nc.vector.tensor_copy(dst, ps[:])
                            else:
                                nc.scalar.activation(
                                    dst, ps[:], mybir.ActivationFunctionType.Copy
                                )
                        nc.sync.dma_start(
                            out=partial[ts_ * 128 : (ts_ + 1) * 128, :],
                            in_=ot[:],
                        )
                        # reduce-scatter each half-batch as soon as its rows
                        # are written, overlapping the rest of the out-proj
                    if do_rs:
                        rs_out = dopool.tile(
                            [TPC, C], bf16, tag="rsout", name=f"rso{b}"
                        )
                        nc.gpsimd.collective_compute(
                            "ReduceScatter",
                            mybir.AluOpType.add,
                            replica_groups=[list(range(NCORES))],
                            ins=[partial[:].opt()],
                            outs=[rs_out[:].opt()],
                        )
                        nc.gpsimd.dma_start(out=outp[b], in_=rs_out[:])
    nc.compile()
    return nc


_NC_CACHE = {}


def _get_nc():
    if "nc" not in _NC_CACHE:
        _NC_CACHE["nc"] = _build()
    return _NC_CACHE["nc"]


def kernel(x, wq, bq, wk, bk, wv, bv, wo, bo):
    x = np.asarray(x, dtype=np.float32)
    wq, bq = np.asarray(wq), np.asarray(bq)
    wk, bk = np.asarray(wk), np.asarray(bk)
    wv, bv = np.asarray(wv), np.asarray(bv)
    wo, bo = np.asarray(wo), np.asarray(bo)
    nc = _get_nc()

    xT_bf = np.ascontiguousarray(x.reshape(TOK, C).T).astype(bfnp)
    in_maps = []
    for c in range(NCORES):
        hs = slice(c * F, (c + 1) * F)
        wqkT_c = np.ascontiguousarray(
            np.concatenate([wq[hs], wk[hs]], axis=0).T
        ).astype(bfnp)
        wvT_c = np.ascontiguousarray(wv[hs].T).astype(bfnp)
        wosT_c = np.ascontiguousarray(wo[:, hs].T).astype(bfnp)
        bqk_c = np.ascontiguousarray(
            np.concatenate([bq[hs], bk[hs]])[:, None]
        ).astype(np.float32)
        in_maps.append(
            {"xT": xT_bf, "wqkT": wqkT_c, "wvT": wvT_c, "wosT": wosT_c, "bqk": bqk_c}
        )
    res = run_bass_kernel_spmd(nc, in_maps, list(range(NCORES)))

    out = np.empty((B, T, C), dtype=np.float32)
    for c in range(NCORES):
        ob = np.asarray(res.results[c]["outp"]).astype(np.float32)  # [B, TPC, C]
        out[:, c * TPC : (c + 1) * TPC, :] = ob
    out += (bo + wo @ bv)[None, None, :]
    return out


# revision 3
# speedup vs baseline: 1.0273x; 1.0003x over previous
import sys

sys.path.insert(0, "/opt/trn_rl_repo")
import numpy as np
import ml_dtypes
import concourse.bass as bass  # noqa: F401
import concourse.mybir as mybir
import concourse.tile as tile
from concourse import bacc
from concourse.bass_utils import run_bass_kernel_spmd

B, T, C, H, D = 4, 2048, 2048, 16, 128
NCORES = 8
HPC = H // NCORES  # 2 heads per core
F = HPC * D  # 256 per-core head features
TOK = B * T  # 8192
CC = C // 128  # 16 contraction chunks for the projections
TBPB = T // 512  # 4 token blocks (512-wide) per batch
KBPB = T // 128  # 16 k blocks (128-wide) per batch
TPC = T // NCORES  # 256 tokens of each batch owned per core after RS

f32 = mybir.dt.float32
bf16 = mybir.dt.bfloat16
SCALE = 1.0 / float(np.sqrt(D))

bfnp = ml_dtypes.bfloat16


def _build(do_attn=True, do_outproj=True, do_rs=True):
    nc = bacc.Bacc(
        "TRN2", target_bir_lowering=False, debug=False, num_devices=NCORES
    )
    xT = nc.dram_tensor("xT", [C, TOK], bf16, kind="ExternalInput")
    wqkT = nc.dram_tensor("wqkT", [C, 2 * F], bf16, kind="ExternalInput")
    wvT = nc.dram_tensor("wvT", [C, F], bf16, kind="ExternalInput")
    wosT = nc.dram_tensor("wosT", [F, C], bf16, kind="ExternalInput")
    bqk = nc.dram_tensor("bqk", [2 * F, 1], f32, kind="ExternalInput")
    outp = nc.dram_tensor("outp", [B, 2, TPC // 2, C], bf16, kind="ExternalOutput")

    with tile.TileContext(nc) as tc:
        with (
            tc.tile_pool(name="const", bufs=1) as cpool,
            tc.tile_pool(name="dram", bufs=2, space="DRAM") as dpool,
            tc.tile_pool(name="dramo", bufs=3, space="DRAM") as dopool,
        ):
            wqk_sb = cpool.tile([128, CC, 2 * F], bf16)
            wv_sb = cpool.tile([128, CC, F], bf16)
            wos_sb = cpool.tile([128, HPC, C], bf16)
            bqk_sb = cpool.tile([128, 4, 1], f32)
            ones_sb = cpool.tile([128, 1], bf16)
            ones1_sb = cpool.tile([1, 128], bf16)
            nc.vector.memset(ones_sb[:], 1.0)
            nc.vector.memset(ones1_sb[:], 1.0)
            nc.sync.dma_start(
                out=wqk_sb[:], in_=wqkT[:].rearrange("(cc p) f -> p cc f", p=128)
            )
            nc.sync.dma_start(
                out=bqk_sb[:], in_=bqk[:].rearrange("(fb p) o -> p fb o", p=128)
            )
            nc.gpsimd.dma_start(
                out=wv_sb[:], in_=wvT[:].rearrange("(cc p) f -> p cc f", p=128)
            )
            nc.gpsimd.dma_start(
                out=wos_sb[:], in_=wosT[:].rearrange("(h p) j -> p h j", p=128)
            )

            with (
                tc.tile_pool(name="px", bufs=3) as px_pool,
                tc.tile_pool(name="qkv", bufs=2) as qkv_pool,
                tc.tile_pool(name="attv", bufs=1) as attv_pool,
                tc.tile_pool(name="pt", bufs=2) as pt_pool,
                tc.tile_pool(name="misc", bufs=3) as misc_pool,
                tc.tile_pool(name="oo", bufs=3) as oo_pool,
                tc.tile_pool(name="psA", bufs=2, space="PSUM") as psA,
                tc.tile_pool(name="psV", bufs=1, space="PSUM") as psV,
                tc.tile_pool(name="ps_st", bufs=2, space="PSUM") as ps_st_pool,
                tc.tile_pool(name="ps_av", bufs=2, space="PSUM") as ps_av_pool,
                tc.tile_pool(name="ps_sum", bufs=1, space="PSUM") as ps_sum_pool,
            ):
                # 4 causal-diagonal masks: delta = kb*128 - qb*512 in {0,128,256,384};
                # mask[kp, qf] = 1 where qf >= kp + delta else 0.
                masks = cpool.tile([128, 4, 512], bf16)
                nc.vector.memset(masks[:], 1.0)
                for mi in range(4):
                    nc.gpsimd.affine_select(
                        out=masks[:, mi],
                        in_=masks[:, mi],
                        compare_op=mybir.AluOpType.is_ge,
                        fill=0.0,
                        base=-mi * 128,
                        pattern=[[1, 512]],
                        channel_multiplier=-1,
                    )
                def load_xt(b, t4, engine):
                    tb = b * TBPB + t4
                    xt = px_pool.tile([128, CC, 512], bf16, tag="xt", name=f"xt{tb}")
                    engine.dma_start(
                        out=xt[:],
                        in_=xT[:, tb * 512 : (tb + 1) * 512].rearrange(
                            "(cc p) t -> p cc t", p=128
                        ),
                    )
                    return xt

                xt_pref = [load_xt(0, t4, nc.sync) for t4 in range(TBPB)]

                for b in range(B):
                    qT_sb = qkv_pool.tile([128, HPC, T], bf16, tag="qT", name=f"qT{b}")
                    kT_sb = qkv_pool.tile([128, HPC, T], bf16, tag="kT", name=f"kT{b}")
                    v_sb = qkv_pool.tile([128, KBPB, F], bf16, tag="v", name=f"v{b}")
                    attv_sb = attv_pool.tile(
                        [128, HPC, T], bf16, tag="attv", name=f"attv{b}"
                    )
                    def proj_block(t4):
                        tb = b * TBPB + t4
                        xt = xt_pref[t4]
                        for fb in range(4):
                            ps = psA.tile([128, 512], f32, tag="psA", name=f"psA{tb}_{fb}")
                            for cc in range(CC):
                                nc.tensor.matmul(
                                    ps[:],
                                    wqk_sb[:, cc, fb * 128 : (fb + 1) * 128],
                                    xt[:, cc],
                                    start=(cc == 0),
                                    stop=(cc == CC - 1),
                                )
                            dst = qT_sb if fb < 2 else kT_sb
                            h = fb % 2
                            nc.vector.tensor_scalar_add(
                                dst[:, h, t4 * 512 : (t4 + 1) * 512],
                                ps[:],
                                bqk_sb[:, fb],
                            )
                            # V group interleaved after each QK group: the QK
                            # matmuls hide the psV-copy latency (bufs=1)
                            sub = fb
                            psv = psV.tile([128, F], f32, tag="psV", name=f"psV{tb}_{sub}")
                            for cc in range(CC):
                                nc.tensor.matmul(
                                    psv[:],
                                    xt[:, cc, sub * 128 : (sub + 1) * 128],
                                    wv_sb[:, cc],
                                    start=(cc == 0),
                                    stop=(cc == CC - 1),
                                )
                            nc.vector.tensor_copy(v_sb[:, t4 * 4 + sub], psv[:])

                    def issue_scores(h, qb):
                        nkb = (qb + 1) * 4
                        q0 = qb * 512
                        pts = []
                        for kb in range(nkb):
                            ps_st = ps_st_pool.tile(
                                [128, 512], f32, tag="st", name=f"st{b}_{h}_{qb}_{kb}"
                            )
                            nc.tensor.matmul(
                                ps_st[:],
                                kT_sb[:, h, kb * 128 : (kb + 1) * 128],
                                qT_sb[:, h, q0 : q0 + 512],
                                start=True,
                                stop=True,
                            )
                            pt = pt_pool.tile(
                                [128, 512], bf16, tag=f"pt{kb}",
                                name=f"pt{b}_{h}_{qb}_{kb}",
                            )
                            nc.scalar.activation(
                                pt[:],
                                ps_st[:],
                                mybir.ActivationFunctionType.Exp,
                                scale=SCALE,
                            )
                            if kb * 128 >= q0:
                                # diagonal: zero where k_global > q_global
                                mi = (kb * 128 - q0) // 128
                                nc.vector.tensor_mul(
                                    pt[:], pt[:], masks[:, mi]
                                )
                            pts.append(pt)
                        return (h, qb, pts)

                    def issue_av(pend):
                        h, qb, pts = pend
                        nkb = len(pts)
                        q0 = qb * 512
                        ps_av = ps_av_pool.tile(
                            [128, 512], f32, tag="av", name=f"av{b}_{h}_{qb}"
                        )
                        ps_sum = ps_sum_pool.tile(
                            [1, 512], f32, tag="sum", name=f"sum{b}_{h}_{qb}"
                        )
                        for kb in range(nkb):
                            nc.tensor.matmul(
                                ps_av[:],
                                v_sb[:, kb, h * 128 : (h + 1) * 128],
                                pts[kb][:],
                                start=(kb == 0),
                                stop=(kb == nkb - 1),
                                skip_group_check=True,
                            )
                            nc.tensor.matmul(
                                ps_sum[:],
                                ones_sb[:],
                                pts[kb][:],
                                start=(kb == 0),
                                stop=(kb == nkb - 1),
                                skip_group_check=True,
                            )
                        recip = misc_pool.tile(
                            [1, 512], bf16, tag="recip", name=f"rc{b}_{h}_{qb}"
                        )
                        with nc.allow_low_precision(
                            reason="bf16 softmax denominators are ample"
                        ):
                            nc.vector.reciprocal(recip[:], ps_sum[:])
                        bc_sb = misc_pool.tile(
                            [128, 512], bf16, tag="bc", name=f"bcs{b}_{h}_{qb}"
                        )
                        nc.gpsimd.partition_broadcast(bc_sb[:], recip[:])
                        nc.vector.tensor_mul(
                            attv_sb[:, h, q0 : q0 + 512], ps_av[:], bc_sb[:]
                        )

                    for t4 in range(TBPB):
                        proj_block(t4)
                    # prefetch next batch's x tiles while attention runs
                    if b + 1 < B:
                        xt_pref = [
                            load_xt(b + 1, t, nc.gpsimd) for t in range(TBPB)
                        ]
                    def outproj_half(hf):
                        # out-projection + reduce-scatter for token rows
                        # [hf*T/2, (hf+1)*T/2). Half 0 is emitted mid-attention
                        # (tokens 0..1023 are fully attended after (h1, qb1)),
                        # so its RS overlaps the rest of the attention phase
                        # and only a half-sized RS remains as the batch tail.
                        # Half 0 rotates psA+st PSUM banks (st is idle there);
                        # half 1 rotates psA+av.
                        partial = dpool.tile(
                            [T // 2, C], bf16, tag=f"part{hf}", name=f"part{b}_{hf}"
                        )
                        for k in range(KBPB // 2):
                            ts_ = hf * (KBPB // 2) + k
                            ot = oo_pool.tile(
                                [128, C], bf16, tag="ot", name=f"ot{b}_{ts_}"
                            )
                            for jb in range(4):
                                i = k * 4 + jb
                                if hf == 0:
                                    pool, tag = (
                                        (ps_st_pool, "st") if i % 4 >= 2 else (psA, "psA")
                                    )
                                else:
                                    pool, tag = (
                                        (ps_av_pool, "av") if i % 4 >= 2 else (psA, "psA")
                                    )
                                ps = pool.tile(
                                    [128, 512], f32, tag=tag, name=f"po{b}_{ts_}_{jb}"
                                )
                                for h in range(HPC):
                                    nc.tensor.matmul(
                                        ps[:],
                                        attv_sb[:, h, ts_ * 128 : (ts_ + 1) * 128],
                                        wos_sb[:, h, jb * 512 : (jb + 1) * 512],
                                        start=(h == 0),
                                        stop=(h == HPC - 1),
                                    )
                                dst = ot[:, jb * 512 : (jb + 1) * 512]
                                # half 0 runs while ACT still drains exps:
                                # keep most copies on DVE there
                                on_act = (i % 4 == 3) if hf == 0 else (i % 2 == 1)
                                if on_act:
                                    nc.scalar.activation(
                                        dst, ps[:], mybir.ActivationFunctionType.Copy
                                    )
                                else:
                                    nc.vector.tensor_copy(dst, ps[:])
                            nc.sync.dma_start(
                                out=partial[k * 128 : (k + 1) * 128, :],
                                in_=ot[:],
                            )
                        if do_rs:
                            rs_out = dopool.tile(
                                [TPC // 2, C], bf16, tag="rsout", name=f"rso{b}_{hf}"
                            )
                            nc.gpsimd.collective_compute(
                                "ReduceScatter",
                                mybir.AluOpType.add,
                                replica_groups=[list(range(NCORES))],
                                ins=[partial[:].opt()],
                                outs=[rs_out[:].opt()],
                            )
                            nc.gpsimd.dma_start(out=outp[b, hf], in_=rs_out[:])

                    pend = None
                    for h in range(HPC if do_attn else 0):
                        for qb in range(TBPB):
                            nxt = issue_scores(h, qb)
                            if pend is not None:
                                issue_av(pend)
                                if do_outproj and pend[0] == 1 and pend[1] == 1:
                                    outproj_half(0)
                            pend = nxt
                    if pend is not None:
                        issue_av(pend)
                    if do_attn and do_outproj:
                        outproj_half(1)
    nc.compile()
    return nc


_NC_CACHE = {}


def _get_nc():
    if "nc" not in _NC_CACHE:
        _NC_CACHE["nc"] = _build()
    return _NC_CACHE["nc"]


def kernel(x, wq, bq, wk, bk, wv, bv, wo, bo):
    x = np.asarray(x, dtype=np.float32)
    wq, bq = np.asarray(wq), np.asarray(bq)
    wk, bk = np.asarray(wk), np.asarray(bk)
    wv, bv = np.asarray(wv), np.asarray(bv)
    wo, bo = np.asarray(wo), np.asarray(bo)
    nc = _get_nc()

    xT_bf = np.ascontiguousarray(x.reshape(TOK, C).T).astype(bfnp)
    in_maps = []
    for c in range(NCORES):
        hs = slice(c * F, (c + 1) * F)
        wqkT_c = np.ascontiguousarray(
            np.concatenate([wq[hs], wk[hs]], axis=0).T
        ).astype(bfnp)
        wvT_c = np.ascontiguousarray(wv[hs].T).astype(bfnp)
        wosT_c = np.ascontiguousarray(wo[:, hs].T).astype(bfnp)
        bqk_c = np.ascontiguousarray(
            np.concatenate([bq[hs], bk[hs]])[:, None]
        ).astype(np.float32)
        in_maps.append(
            {"xT": xT_bf, "wqkT": wqkT_c, "wvT": wvT_c, "wosT": wosT_c, "bqk": bqk_c}
        )
    res = run_bass_kernel_spmd(nc, in_maps, list(range(NCORES)))

    out = np.empty((B, T, C), dtype=np.float32)
    for c in range(NCORES):
        ob = np.asarray(res.results[c]["outp"]).astype(np.float32)  # [B,2,TPC/2,C]
        for hf in range(2):
            r0 = hf * (T // 2) + c * (TPC // 2)
            out[:, r0 : r0 + TPC // 2, :] = ob[:, hf]
    out += (bo + wo @ bv)[None, None, :]
    return out


# revision 4
# speedup vs baseline: 1.0562x; 1.0281x over previous
import sys

sys.path.insert(0, "/opt/trn_rl_repo")
import numpy as np
import ml_dtypes
import concourse.bass as bass  # noqa: F401
import concourse.mybir as mybir
import concourse.tile as tile
from concourse import bacc
from concourse.bass_utils import run_bass_kernel_spmd

B, T, C, H, D = 4, 2048, 2048, 16, 128
NCORES = 8
HPC = H // NCORES  # 2 heads per core
F = HPC * D  # 256 per-core head features
TOK = B * T  # 8192
CC = C // 128  # 16 contraction chunks for the projections
TBPB = T // 512  # 4 token blocks (512-wide) per batch
KBPB = T // 128  # 16 k blocks (128-wide) per batch
TPC = T // NCORES  # 256 tokens of each batch owned per core after RS

f32 = mybir.dt.float32
bf16 = mybir.dt.bfloat16
SCALE = 1.0 / float(np.sqrt(D))

bfnp = ml_dtypes.bfloat16


def _build(do_attn=True, do_outproj=True, do_rs=True):
    nc = bacc.Bacc(
        "TRN2", target_bir_lowering=False, debug=False, num_devices=NCORES
    )
    xT = nc.dram_tensor("xT", [C, TOK], bf16, kind="ExternalInput")
    wqkT = nc.dram_tensor("wqkT", [C, 2 * F], bf16, kind="ExternalInput")
    wvT = nc.dram_tensor("wvT", [C, F], bf16, kind="ExternalInput")
    wosT = nc.dram_tensor("wosT", [F, C], bf16, kind="ExternalInput")
    bqk = nc.dram_tensor("bqk", [2 * F, 1], f32, kind="ExternalInput")
    outp = nc.dram_tensor("outp", [B, 2, TPC // 2, C], bf16, kind="ExternalOutput")

    with tile.TileContext(nc) as tc:
        with (
            tc.tile_pool(name="const", bufs=1) as cpool,
            tc.tile_pool(name="dram", bufs=2, space="DRAM") as dpool,
            tc.tile_pool(name="dramo", bufs=3, space="DRAM") as dopool,
        ):
            wqk_sb = cpool.tile([128, CC, 2 * F], bf16)
            wv_sb = cpool.tile([128, CC, F], bf16)
            wos_sb = cpool.tile([128, HPC, C], bf16)
            bqk_sb = cpool.tile([128, 4, 1], f32)
            ones_sb = cpool.tile([128, 1], bf16)
            ones1_sb = cpool.tile([1, 128], bf16)
            nc.vector.memset(ones_sb[:], 1.0)
            nc.vector.memset(ones1_sb[:], 1.0)
            nc.sync.dma_start(
                out=wqk_sb[:], in_=wqkT[:].rearrange("(cc p) f -> p cc f", p=128)
            )
            nc.sync.dma_start(
                out=bqk_sb[:], in_=bqk[:].rearrange("(fb p) o -> p fb o", p=128)
            )
            nc.gpsimd.dma_start(
                out=wv_sb[:], in_=wvT[:].rearrange("(cc p) f -> p cc f", p=128)
            )
            nc.gpsimd.dma_start(
                out=wos_sb[:], in_=wosT[:].rearrange("(h p) j -> p h j", p=128)
            )

            with (
                tc.tile_pool(name="px", bufs=3) as px_pool,
                tc.tile_pool(name="qkv", bufs=2) as qkv_pool,
                tc.tile_pool(name="attv", bufs=1) as attv_pool,
                tc.tile_pool(name="pt", bufs=2) as pt_pool,
                tc.tile_pool(name="misc", bufs=3) as misc_pool,
                tc.tile_pool(name="oo", bufs=3) as oo_pool,
                tc.tile_pool(name="psA", bufs=2, space="PSUM") as psA,
                tc.tile_pool(name="psV", bufs=1, space="PSUM") as psV,
                tc.tile_pool(name="ps_st", bufs=2, space="PSUM") as ps_st_pool,
                tc.tile_pool(name="ps_av", bufs=2, space="PSUM") as ps_av_pool,
                tc.tile_pool(name="ps_sum", bufs=1, space="PSUM") as ps_sum_pool,
            ):
                # 4 causal-diagonal masks: delta = kb*128 - qb*512 in {0,128,256,384};
                # mask[kp, qf] = 1 where qf >= kp + delta else 0.
                masks = cpool.tile([128, 4, 512], bf16)
                nc.vector.memset(masks[:], 1.0)
                for mi in range(4):
                    nc.gpsimd.affine_select(
                        out=masks[:, mi],
                        in_=masks[:, mi],
                        compare_op=mybir.AluOpType.is_ge,
                        fill=0.0,
                        base=-mi * 128,
                        pattern=[[1, 512]],
                        channel_multiplier=-1,
                    )
                def load_xt(b, t4, engine):
                    tb = b * TBPB + t4
                    xt = px_pool.tile([128, CC, 512], bf16, tag="xt", name=f"xt{tb}")
                    engine.dma_start(
                        out=xt[:],
                        in_=xT[:, tb * 512 : (tb + 1) * 512].rearrange(
                            "(cc p) t -> p cc t", p=128
                        ),
                    )
                    return xt

                xt_pref = [load_xt(0, t4, nc.sync) for t4 in range(TBPB)]

                for b in range(B):
                    qT_sb = qkv_pool.tile([128, HPC, T], bf16, tag="qT", name=f"qT{b}")
                    kT_sb = qkv_pool.tile([128, HPC, T], bf16, tag="kT", name=f"kT{b}")
                    v_sb = qkv_pool.tile([128, KBPB, F], bf16, tag="v", name=f"v{b}")
                    attv_sb = attv_pool.tile(
                        [128, HPC, T], bf16, tag="attv", name=f"attv{b}"
                    )
                    def proj_block(t4):
                        tb = b * TBPB + t4
                        xt = xt_pref[t4]
                        for fb in range(4):
                            ps = psA.tile([128, 512], f32, tag="psA", name=f"psA{tb}_{fb}")
                            for cc in range(CC):
                                nc.tensor.matmul(
                                    ps[:],
                                    wqk_sb[:, cc, fb * 128 : (fb + 1) * 128],
                                    xt[:, cc],
                                    start=(cc == 0),
                                    stop=(cc == CC - 1),
                                )
                            dst = qT_sb if fb < 2 else kT_sb
                            h = fb % 2
                            nc.vector.tensor_scalar_add(
                                dst[:, h, t4 * 512 : (t4 + 1) * 512],
                                ps[:],
                                bqk_sb[:, fb],
                            )
                            # V group interleaved after each QK group: the QK
                            # matmuls hide the psV-copy latency (bufs=1)
                            sub = fb
                            psv = psV.tile([128, F], f32, tag="psV", name=f"psV{tb}_{sub}")
                            for cc in range(CC):
                                nc.tensor.matmul(
                                    psv[:],
                                    xt[:, cc, sub * 128 : (sub + 1) * 128],
                                    wv_sb[:, cc],
                                    start=(cc == 0),
                                    stop=(cc == CC - 1),
                                )
                            nc.vector.tensor_copy(v_sb[:, t4 * 4 + sub], psv[:])

                    def issue_scores(h, qb):
                        nkb = (qb + 1) * 4
                        q0 = qb * 512
                        pts = []
                        for kb in range(nkb):
                            ps_st = ps_st_pool.tile(
                                [128, 512], f32, tag="st", name=f"st{b}_{h}_{qb}_{kb}"
                            )
                            nc.tensor.matmul(
                                ps_st[:],
                                kT_sb[:, h, kb * 128 : (kb + 1) * 128],
                                qT_sb[:, h, q0 : q0 + 512],
                                start=True,
                                stop=True,
                            )
                            pt = pt_pool.tile(
                                [128, 512], bf16, tag=f"pt{kb}",
                                name=f"pt{b}_{h}_{qb}_{kb}",
                            )
                            nc.scalar.activation(
                                pt[:],
                                ps_st[:],
                                mybir.ActivationFunctionType.Exp,
                                scale=SCALE,
                            )
                            if kb * 128 >= q0:
                                # diagonal: zero where k_global > q_global
                                mi = (kb * 128 - q0) // 128
                                nc.vector.tensor_mul(
                                    pt[:], pt[:], masks[:, mi]
                                )
                            pts.append(pt)
                        return (h, qb, pts)

                    def issue_av(pend):
                        h, qb, pts = pend
                        nkb = len(pts)
                        q0 = qb * 512
                        ps_av = ps_av_pool.tile(
                            [128, 512], f32, tag="av", name=f"av{b}_{h}_{qb}"
                        )
                        ps_sum = ps_sum_pool.tile(
                            [1, 512], f32, tag="sum", name=f"sum{b}_{h}_{qb}"
                        )
                        for kb in range(nkb):
                            nc.tensor.matmul(
                                ps_av[:],
                                v_sb[:, kb, h * 128 : (h + 1) * 128],
                                pts[kb][:],
                                start=(kb == 0),
                                stop=(kb == nkb - 1),
                                skip_group_check=True,
                            )
                            nc.tensor.matmul(
                                ps_sum[:],
                                ones_sb[:],
                                pts[kb][:],
                                start=(kb == 0),
                                stop=(kb == nkb - 1),
                                skip_group_check=True,
                            )
                        recip = misc_pool.tile(
                            [1, 512], bf16, tag="recip", name=f"rc{b}_{h}_{qb}"
                        )
                        with nc.allow_low_precision(
                            reason="bf16 softmax denominators are ample"
                        ):
                            nc.vector.reciprocal(recip[:], ps_sum[:])
                        bc_sb = misc_pool.tile(
                            [128, 512], bf16, tag="bc", name=f"bcs{b}_{h}_{qb}"
                        )
                        nc.gpsimd.partition_broadcast(bc_sb[:], recip[:])
                        nc.vector.tensor_mul(
                            attv_sb[:, h, q0 : q0 + 512], ps_av[:], bc_sb[:]
                        )

                    for t4 in range(TBPB):
                        proj_block(t4)
                    # prefetch next batch's x tiles while attention runs
                    if b + 1 < B:
                        xt_pref = [
                            load_xt(b + 1, t, nc.gpsimd) for t in range(TBPB)
                        ]
                    def outproj_half(hf):
                        # out-projection + reduce-scatter for token rows
                        # [hf*T/2, (hf+1)*T/2). Half 0 is emitted mid-attention
                        # (tokens 0..1023 are fully attended after (h1, qb1)),
                        # so its RS overlaps the rest of the attention phase
                        # and only a half-sized RS remains as the batch tail.
                        # Half 0 rotates psA+st PSUM banks (st is idle there);
                        # half 1 rotates psA+av.
                        partial = dpool.tile(
                            [T // 2, C], bf16, tag=f"part{hf}", name=f"part{b}_{hf}"
                        )
                        for k in range(KBPB // 2):
                            ts_ = hf * (KBPB // 2) + k
                            ot = oo_pool.tile(
                                [128, C], bf16, tag="ot", name=f"ot{b}_{ts_}"
                            )
                            for jb in range(4):
                                i = k * 4 + jb
                                if hf == 0:
                                    pool, tag = (
                                        (ps_st_pool, "st") if i % 4 >= 2 else (psA, "psA")
                                    )
                                else:
                                    pool, tag = (
                                        (ps_av_pool, "av") if i % 4 >= 2 else (psA, "psA")
                                    )
                                ps = pool.tile(
                                    [128, 512], f32, tag=tag, name=f"po{b}_{ts_}_{jb}"
                                )
                                for h in range(HPC):
                                    nc.tensor.matmul(
                                        ps[:],
                                        attv_sb[:, h, ts_ * 128 : (ts_ + 1) * 128],
                                        wos_sb[:, h, jb * 512 : (jb + 1) * 512],
                                        start=(h == 0),
                                        stop=(h == HPC - 1),
                                    )
                                dst = ot[:, jb * 512 : (jb + 1) * 512]
                                # ACT drains exp backlogs at the start of both
                                # halves: keep early copies on DVE
                                on_act = (i % 4 == 3) if hf == 0 else (
                                    i >= 8 and i % 2 == 1
                                )
                                if on_act:
                                    nc.scalar.activation(
                                        dst, ps[:], mybir.ActivationFunctionType.Copy
                                    )
                                else:
                                    nc.vector.tensor_copy(dst, ps[:])
                            nc.sync.dma_start(
                                out=partial[k * 128 : (k + 1) * 128, :],
                                in_=ot[:],
                            )
                        if do_rs:
                            rs_out = dopool.tile(
                                [TPC // 2, C], bf16, tag="rsout", name=f"rso{b}_{hf}"
                            )
                            nc.gpsimd.collective_compute(
                                "ReduceScatter",
                                mybir.AluOpType.add,
                                replica_groups=[list(range(NCORES))],
                                ins=[partial[:].opt()],
                                outs=[rs_out[:].opt()],
                            )
                            nc.gpsimd.dma_start(out=outp[b, hf], in_=rs_out[:])

                    pend = None
                    for h in range(HPC if do_attn else 0):
                        for qb in range(TBPB):
                            nxt = issue_scores(h, qb)
                            if pend is not None:
                                issue_av(pend)
                                if do_outproj and pend[0] == 1 and pend[1] == 1:
                                    outproj_half(0)
                            pend = nxt
                    if pend is not None:
                        issue_av(pend)
                    if do_attn and do_outproj:
                        outproj_half(1)
    nc.compile()
    return nc


_NC_CACHE = {}


def _get_nc():
    if "nc" not in _NC_CACHE:
        _NC_CACHE["nc"] = _build()
    return _NC_CACHE["nc"]


def kernel(x, wq, bq, wk, bk, wv, bv, wo, bo):
    x = np.asarray(x, dtype=np.float32)
    wq, bq = np.asarray(wq), np.asarray(bq)
    wk, bk = np.asarray(wk), np.asarray(bk)
    wv, bv = np.asarray(wv), np.asarray(bv)
    wo, bo = np.asarray(wo), np.asarray(bo)
    nc = _get_nc()

    xT_bf = np.ascontiguousarray(x.reshape(TOK, C).T).astype(bfnp)
    in_maps = []
    for c in range(NCORES):
        hs = slice(c * F, (c + 1) * F)
        wqkT_c = np.ascontiguousarray(
            np.concatenate([wq[hs], wk[hs]], axis=0).T
        ).astype(bfnp)
        wvT_c = np.ascontiguousarray(wv[hs].T).astype(bfnp)
        wosT_c = np.ascontiguousarray(wo[:, hs].T).astype(bfnp)
        bqk_c = np.ascontiguousarray(
            np.concatenate([bq[hs], bk[hs]])[:, None]
        ).astype(np.float32)
        in_maps.append(
            {"xT": xT_bf, "wqkT": wqkT_c, "wvT": wvT_c, "wosT": wosT_c, "bqk": bqk_c}
        )
    res = run_bass_kernel_spmd(nc, in_maps, list(range(NCORES)))

    out = np.empty((B, T, C), dtype=np.float32)
    for c in range(NCORES):
        ob = np.asarray(res.results[c]["outp"]).astype(np.float32)  # [B,2,TPC/2,C]
        for hf in range(2):
            r0 = hf * (T // 2) + c * (TPC // 2)
            out[:, r0 : r0 + TPC // 2, :] = ob[:, hf]
    out += (bo + wo @ bv)[None, None, :]
    return out


# revision 5
# speedup vs baseline: 1.0725x; 1.0154x over previous
import sys

sys.path.insert(0, "/opt/trn_rl_repo")
import numpy as np
import ml_dtypes
import concourse.bass as bass  # noqa: F401
import concourse.mybir as mybir
import concourse.tile as tile
from concourse import bacc
from concourse.bass_utils import run_bass_kernel_spmd

B, T, C, H, D = 4, 2048, 2048, 16, 128
NCORES = 8
HPC = H // NCORES  # 2 heads per core
F = HPC * D  # 256 per-core head features
TOK = B * T  # 8192
CC = C // 128  # 16 contraction chunks for the projections
TBPB = T // 512  # 4 token blocks (512-wide) per batch
KBPB = T // 128  # 16 k blocks (128-wide) per batch
TPC = T // NCORES  # 256 tokens of each batch owned per core after RS

f32 = mybir.dt.float32
bf16 = mybir.dt.bfloat16
SCALE = 1.0 / float(np.sqrt(D))

bfnp = ml_dtypes.bfloat16


def _build(do_attn=True, do_outproj=True, do_rs=True):
    nc = bacc.Bacc(
        "TRN2", target_bir_lowering=False, debug=False, num_devices=NCORES
    )
    xT = nc.dram_tensor("xT", [C, TOK], bf16, kind="ExternalInput")
    wqkT = nc.dram_tensor("wqkT", [C, 2 * F], bf16, kind="ExternalInput")
    wvT = nc.dram_tensor("wvT", [C, F], bf16, kind="ExternalInput")
    wosT = nc.dram_tensor("wosT", [F, C], bf16, kind="ExternalInput")
    bqk = nc.dram_tensor("bqk", [2 * F, 1], f32, kind="ExternalInput")
    outp = nc.dram_tensor("outp", [B, 2, TPC // 2, C], bf16, kind="ExternalOutput")

    with tile.TileContext(nc) as tc:
        with (
            tc.tile_pool(name="const", bufs=1) as cpool,
            tc.tile_pool(name="dram", bufs=2, space="DRAM") as dpool,
            tc.tile_pool(name="dramo", bufs=3, space="DRAM") as dopool,
        ):
            wqk_sb = cpool.tile([128, CC, 2 * F], bf16)
            wv_sb = cpool.tile([128, CC, F], bf16)
            wos_sb = cpool.tile([128, HPC, C], bf16)
            bqk_sb = cpool.tile([128, 4, 1], f32)
            ones_sb = cpool.tile([128, 1], bf16)
            ones1_sb = cpool.tile([1, 128], bf16)
            nc.vector.memset(ones_sb[:], 1.0)
            nc.vector.memset(ones1_sb[:], 1.0)
            nc.sync.dma_start(
                out=wqk_sb[:], in_=wqkT[:].rearrange("(cc p) f -> p cc f", p=128)
            )
            nc.sync.dma_start(
                out=bqk_sb[:], in_=bqk[:].rearrange("(fb p) o -> p fb o", p=128)
            )
            nc.gpsimd.dma_start(
                out=wv_sb[:], in_=wvT[:].rearrange("(cc p) f -> p cc f", p=128)
            )
            nc.gpsimd.dma_start(
                out=wos_sb[:], in_=wosT[:].rearrange("(h p) j -> p h j", p=128)
            )

            with (
                tc.tile_pool(name="px", bufs=3) as px_pool,
                tc.tile_pool(name="qkv", bufs=2) as qkv_pool,
                tc.tile_pool(name="attv", bufs=1) as attv_pool,
                tc.tile_pool(name="pt", bufs=2) as pt_pool,
                tc.tile_pool(name="misc", bufs=3) as misc_pool,
                tc.tile_pool(name="oo", bufs=3) as oo_pool,
                tc.tile_pool(name="psA", bufs=2, space="PSUM") as psA,
                tc.tile_pool(name="psV", bufs=1, space="PSUM") as psV,
                tc.tile_pool(name="ps_st", bufs=2, space="PSUM") as ps_st_pool,
                tc.tile_pool(name="ps_av", bufs=2, space="PSUM") as ps_av_pool,
                tc.tile_pool(name="ps_sum", bufs=1, space="PSUM") as ps_sum_pool,
            ):
                # 4 causal-diagonal masks: delta = kb*128 - qb*512 in {0,128,256,384};
                # mask[kp, qf] = 1 where qf >= kp + delta else 0.
                masks = cpool.tile([128, 4, 512], bf16)
                nc.vector.memset(masks[:], 1.0)
                for mi in range(4):
                    nc.gpsimd.affine_select(
                        out=masks[:, mi],
                        in_=masks[:, mi],
                        compare_op=mybir.AluOpType.is_ge,
                        fill=0.0,
                        base=-mi * 128,
                        pattern=[[1, 512]],
                        channel_multiplier=-1,
                    )
                def load_xt(b, t4, engine):
                    tb = b * TBPB + t4
                    xt = px_pool.tile([128, CC, 512], bf16, tag="xt", name=f"xt{tb}")
                    engine.dma_start(
                        out=xt[:],
                        in_=xT[:, tb * 512 : (tb + 1) * 512].rearrange(
                            "(cc p) t -> p cc t", p=128
                        ),
                    )
                    return xt

                xt_pref = [load_xt(0, t4, nc.sync) for t4 in range(TBPB)]

                for b in range(B):
                    qT_sb = qkv_pool.tile([128, HPC, T], bf16, tag="qT", name=f"qT{b}")
                    kT_sb = qkv_pool.tile([128, HPC, T], bf16, tag="kT", name=f"kT{b}")
                    v_sb = qkv_pool.tile([128, KBPB, F], bf16, tag="v", name=f"v{b}")
                    attv_sb = attv_pool.tile(
                        [128, HPC, T], bf16, tag="attv", name=f"attv{b}"
                    )
                    def proj_block(t4):
                        tb = b * TBPB + t4
                        xt = xt_pref[t4]
                        for fb in range(4):
                            ps = psA.tile([128, 512], f32, tag="psA", name=f"psA{tb}_{fb}")
                            for cc in range(CC):
                                nc.tensor.matmul(
                                    ps[:],
                                    wqk_sb[:, cc, fb * 128 : (fb + 1) * 128],
                                    xt[:, cc],
                                    start=(cc == 0),
                                    stop=(cc == CC - 1),
                                )
                            dst = qT_sb if fb < 2 else kT_sb
                            h = fb % 2
                            nc.vector.tensor_scalar_add(
                                dst[:, h, t4 * 512 : (t4 + 1) * 512],
                                ps[:],
                                bqk_sb[:, fb],
                            )
                            # V group interleaved after each QK group: the QK
                            # matmuls hide the psV-copy latency (bufs=1)
                            sub = fb
                            psv = psV.tile([128, F], f32, tag="psV", name=f"psV{tb}_{sub}")
                            for cc in range(CC):
                                nc.tensor.matmul(
                                    psv[:],
                                    xt[:, cc, sub * 128 : (sub + 1) * 128],
                                    wv_sb[:, cc],
                                    start=(cc == 0),
                                    stop=(cc == CC - 1),
                                )
                            nc.vector.tensor_copy(v_sb[:, t4 * 4 + sub], psv[:])

                    def issue_scores(h, qb):
                        nkb = (qb + 1) * 4
                        q0 = qb * 512
                        pts = []
                        for kb in range(nkb):
                            ps_st = ps_st_pool.tile(
                                [128, 512], f32, tag="st", name=f"st{b}_{h}_{qb}_{kb}"
                            )
                            nc.tensor.matmul(
                                ps_st[:],
                                kT_sb[:, h, kb * 128 : (kb + 1) * 128],
                                qT_sb[:, h, q0 : q0 + 512],
                                start=True,
                                stop=True,
                            )
                            pt = pt_pool.tile(
                                [128, 512], bf16, tag=f"pt{kb}",
                                name=f"pt{b}_{h}_{qb}_{kb}",
                            )
                            nc.scalar.activation(
                                pt[:],
                                ps_st[:],
                                mybir.ActivationFunctionType.Exp,
                                scale=SCALE,
                            )
                            if kb * 128 >= q0:
                                # diagonal: zero where k_global > q_global
                                mi = (kb * 128 - q0) // 128
                                nc.vector.tensor_mul(
                                    pt[:], pt[:], masks[:, mi]
                                )
                            pts.append(pt)
                        return (h, qb, pts)

                    def issue_av(pend):
                        h, qb, pts = pend
                        nkb = len(pts)
                        q0 = qb * 512
                        ps_av = ps_av_pool.tile(
                            [128, 512], f32, tag="av", name=f"av{b}_{h}_{qb}"
                        )
                        ps_sum = ps_sum_pool.tile(
                            [1, 512], f32, tag="sum", name=f"sum{b}_{h}_{qb}"
                        )
                        for kb in range(nkb):
                            nc.tensor.matmul(
                                ps_av[:],
                                v_sb[:, kb, h * 128 : (h + 1) * 128],
                                pts[kb][:],
                                start=(kb == 0),
                                stop=(kb == nkb - 1),
                                skip_group_check=True,
                            )
                            if kb % 2 == 1:
                                # pair-reduce exp tiles on the DVE (bf16, 2x
                                # mode) so the PE pays one ones-matmul per
                                # PAIR of k blocks instead of per block
                                k2 = kb // 2
                                sp = misc_pool.tile(
                                    [128, 512], bf16, tag=f"sp{k2 % 2}",
                                    name=f"sp{b}_{h}_{qb}_{k2}",
                                )
                                nc.vector.tensor_add(
                                    sp[:], pts[kb - 1][:], pts[kb][:]
                                )
                                nc.tensor.matmul(
                                    ps_sum[:],
                                    ones_sb[:],
                                    sp[:],
                                    start=(k2 == 0),
                                    stop=(k2 == nkb // 2 - 1),
                                    skip_group_check=True,
                                )
                        recip = misc_pool.tile(
                            [1, 512], bf16, tag="recip", name=f"rc{b}_{h}_{qb}"
                        )
                        with nc.allow_low_precision(
                            reason="bf16 softmax denominators are ample"
                        ):
                            nc.vector.reciprocal(recip[:], ps_sum[:])
                        bc_sb = misc_pool.tile(
                            [128, 512], bf16, tag="bc", name=f"bcs{b}_{h}_{qb}"
                        )
                        nc.gpsimd.partition_broadcast(bc_sb[:], recip[:])
                        nc.vector.tensor_mul(
                            attv_sb[:, h, q0 : q0 + 512], ps_av[:], bc_sb[:]
                        )

                    for t4 in range(TBPB):
                        proj_block(t4)
                    # prefetch next batch's x tiles while attention runs
                    if b + 1 < B:
                        xt_pref = [
                            load_xt(b + 1, t, nc.gpsimd) for t in range(TBPB)
                        ]
                    def outproj_half(hf):
                        # out-projection + reduce-scatter for token rows
                        # [hf*T/2, (hf+1)*T/2). Half 0 is emitted mid-attention
                        # (tokens 0..1023 are fully attended after (h1, qb1)),
                        # so its RS overlaps the rest of the attention phase
                        # and only a half-sized RS remains as the batch tail.
                        # Half 0 rotates psA+st PSUM banks (st is idle there);
                        # half 1 rotates psA+av.
                        partial = dpool.tile(
                            [T // 2, C], bf16, tag=f"part{hf}", name=f"part{b}_{hf}"
                        )
                        for k in range(KBPB // 2):
                            ts_ = hf * (KBPB // 2) + k
                            ot = oo_pool.tile(
                                [128, C], bf16, tag="ot", name=f"ot{b}_{ts_}"
                            )
                            for jb in range(4):
                                i = k * 4 + jb
                                if hf == 0:
                                    pool, tag = (
                                        (ps_st_pool, "st") if i % 4 >= 2 else (psA, "psA")
                                    )
                                else:
                                    pool, tag = (
                                        (ps_av_pool, "av") if i % 4 >= 2 else (psA, "psA")
                                    )
                                ps = pool.tile(
                                    [128, 512], f32, tag=tag, name=f"po{b}_{ts_}_{jb}"
                                )
                                for h in range(HPC):
                                    nc.tensor.matmul(
                                        ps[:],
                                        attv_sb[:, h, ts_ * 128 : (ts_ + 1) * 128],
                                        wos_sb[:, h, jb * 512 : (jb + 1) * 512],
                                        start=(h == 0),
                                        stop=(h == HPC - 1),
                                    )
                                dst = ot[:, jb * 512 : (jb + 1) * 512]
                                # ACT drains exp backlogs at the start of both
                                # halves: keep early copies on DVE
                                on_act = (i % 4 == 3) if hf == 0 else (
                                    i >= 8 and i % 2 == 1
                                )
                                if on_act:
                                    nc.scalar.activation(
                                        dst, ps[:], mybir.ActivationFunctionType.Copy
                                    )
                                else:
                                    nc.vector.tensor_copy(dst, ps[:])
                            nc.sync.dma_start(
                                out=partial[k * 128 : (k + 1) * 128, :],
                                in_=ot[:],
                            )
                        if do_rs:
                            rs_out = dopool.tile(
                                [TPC // 2, C], bf16, tag="rsout", name=f"rso{b}_{hf}"
                            )
                            nc.gpsimd.collective_compute(
                                "ReduceScatter",
                                mybir.AluOpType.add,
                                replica_groups=[list(range(NCORES))],
                                ins=[partial[:].opt()],
                                outs=[rs_out[:].opt()],
                            )
                            nc.gpsimd.dma_start(out=outp[b, hf], in_=rs_out[:])

                    pend = None
                    for h in range(HPC if do_attn else 0):
                        for qb in range(TBPB):
                            nxt = issue_scores(h, qb)
                            if pend is not None:
                                issue_av(pend)
                                if do_outproj and pend[0] == 1 and pend[1] == 1:
                                    outproj_half(0)
                            pend = nxt
                    if pend is not None:
                        issue_av(pend)
                    if do_attn and do_outproj:
                        outproj_half(1)
    nc.compile()
    return nc


_NC_CACHE = {}


def _get_nc():
    if "nc" not in _NC_CACHE:
        _NC_CACHE["nc"] = _build()
    return _NC_CACHE["nc"]


def kernel(x, wq, bq, wk, bk, wv, bv, wo, bo):
    x = np.asarray(x, dtype=np.float32)
    wq, bq = np.asarray(wq), np.asarray(bq)
    wk, bk = np.asarray(wk), np.asarray(bk)
    wv, bv = np.asarray(wv), np.asarray(bv)
    wo, bo = np.asarray(wo), np.asarray(bo)
    nc = _get_nc()

    xT_bf = np.ascontiguousarray(x.reshape(TOK, C).T).astype(bfnp)
    in_maps = []
    for c in range(NCORES):
        hs = slice(c * F, (c + 1) * F)
        wqkT_c = np.ascontiguousarray(
            np.concatenate([wq[hs], wk[hs]], axis=0).T
        ).astype(bfnp)
        wvT_c = np.ascontiguousarray(wv[hs].T).astype(bfnp)
        wosT_c = np.ascontiguousarray(wo[:, hs].T).astype(bfnp)
        bqk_c = np.ascontiguousarray(
            np.concatenate([bq[hs], bk[hs]])[:, None]
        ).astype(np.float32)
        in_maps.append(
            {"xT": xT_bf, "wqkT": wqkT_c, "wvT": wvT_c, "wosT": wosT_c, "bqk": bqk_c}
        )
    res = run_bass_kernel_spmd(nc, in_maps, list(range(NCORES)))

    out = np.empty((B, T, C), dtype=np.float32)
    for c in range(NCORES):
        ob = np.asarray(res.results[c]["outp"]).astype(np.float32)  # [B,2,TPC/2,C]
        for hf in range(2):
            r0 = hf * (T // 2) + c * (TPC // 2)
            out[:, r0 : r0 + TPC // 2, :] = ob[:, hf]
    out += (bo + wo @ bv)[None, None, :]
    return out
